# revision 48
# baseline (speedup 1.0000x reference)
"""Trainium2 Bass kernel for nn_Cell_A (capsule cell: conv1d -> squash ->
strided conv2d -> 3-iter dynamic routing).

Sharding: pure data parallel, batch B=8 across 8 NeuronCores.

v3 design (stall-free pipelined rewrite of v2's 437us):
 - 2-superchunk staggered routing: while sc A's softmax/mt2 build runs on
   DVE/GpSimd/ACT, the PE runs sc B's P-op (and vice versa) -> no PE
   dependency stalls, HAM stays at 2.4 GHz.
 - everything split per l-chunk (j): shorter recurrence, fully contiguous
   PE rhs slices (N=256 s-op groups).
 - b-state kept in PSUM; r2 softmax via exp(P1+P2)=exp(P1)*exp(P2)
   (et2 = et1 * exp(P2)) -> no bt add, fewer PSUM banks.
 - softmax/squash elementwise ops moved to the idle GpSimd engine
   (zt/ct/s2/sqv/STT/f); DVE only does mt/mt2/vprev/recip/et2.
 - conv1 tap-paired: K=128 (two taps stacked on partitions), 9 taps in 5
   matmuls; bias folded into the ACT psum evacuation.
 - s0 for all 4 superchunks + their r1 squash prefilled during the
   ytap3 DRAM-bounce window.
"""

import os
import sys

import numpy as np

sys.path.insert(0, "/opt/trn_rl_repo")

K, L = 64, 1024
CP, APd, G2 = 32, 8, 9
CSA, ASA, G3 = 16, 16, 3
NO = CSA * ASA          # 256 conv2 out channels
EPS = 1e-8
NCORES = 8
NSC = 4                 # superchunks (256 l each)
PL = 128                # l per chunk

_CACHE = {}
_B2_IS_ZERO = [True]


def _build_nc():
    import concourse.bacc as bacc
    import concourse.mybir as mybir
    import concourse.tile as tile
    from concourse.mybir import ActivationFunctionType as AF, AluOpType as OP

    f32 = mybir.dt.float32
    bf16 = mybir.dt.bfloat16

    # Pin all ACT activations to the one table set containing Exp+Ln+Copy so
    # the table-load pass emits a single hoisted load (no per-call reloads).
    from concourse.hw_specs import get_activation_tables as _gat
    _keep = "natural_log_exp_and_others"
    _used = {AF.Exp, AF.Ln, AF.Copy, AF.Identity, AF.Square}

    def _gat_one(arch):
        tabs = _gat(arch)
        assert _used <= tabs[_keep]
        return {n: (f if n == _keep else (f - _used)) for n, f in tabs.items()}

    bacc.get_activation_tables = _gat_one

    nc = bacc.Bacc("TRN2", target_bir_lowering=False, debug=False)

    W = L + 8
    xin_d = nc.dram_tensor("xin", [K, L], bf16, kind="ExternalInput")
    w1p_d = nc.dram_tensor("w1p", [128, 5, 2, 128], bf16, kind="ExternalInput")
    b1t_d = nc.dram_tensor("b1t", [128, 2], f32, kind="ExternalInput")
    w2blk_d = nc.dram_tensor("w2blk", [48, 2, NO], bf16, kind="ExternalInput")
    wfold_d = nc.dram_tensor("wfold", [128, 2, G3, NO], bf16, kind="ExternalInput")
    blk8_d = nc.dram_tensor("blk8", [128, 128], bf16, kind="ExternalInput")
    iden_d = nc.dram_tensor("iden", [128, 128], bf16, kind="ExternalInput")
    b2f_d = nc.dram_tensor("b2f", [NO], f32, kind="ExternalInput")  # 2*b2 perm
    b2p_d = nc.dram_tensor("b2p", [NO], f32, kind="ExternalInput")  # b2 perm
    out_d = nc.dram_tensor("out", [L * CSA, ASA], f32, kind="ExternalOutput")
    # scratch for the ysq partition-rearrange bounce (rows = cp*8+a)
    ydram_d = nc.dram_tensor("ydram", [256, L + 2], bf16, kind="Internal")

    out_v = out_d.ap().rearrange("(l s) a -> l s a", s=CSA)

    with tile.TileContext(nc) as tc:
        import contextlib
        ctx = contextlib.ExitStack()
        with ctx:
            singles = ctx.enter_context(tc.tile_pool(name="singles", bufs=1))
            vpool = ctx.enter_context(tc.tile_pool(name="vpool", bufs=3))
            mtx = ctx.enter_context(tc.tile_pool(name="mtx", bufs=2))
            sm = ctx.enter_context(tc.tile_pool(name="sm", bufs=4))
            sqa = ctx.enter_context(tc.tile_pool(name="sqa", bufs=4))
            vout = ctx.enter_context(tc.tile_pool(name="vout", bufs=2))
            ps = ctx.enter_context(tc.tile_pool(name="ps", bufs=3, space="PSUM"))
            ps_s = ctx.enter_context(tc.tile_pool(name="ps_s", bufs=1, space="PSUM"))
            ps_p = ctx.enter_context(tc.tile_pool(name="ps_p", bufs=2, space="PSUM"))

            # ---- constants / weights ----
            xpad2 = singles.tile([128, W], bf16)
            nc.vector.memset(xpad2[0:K, 0:4], 0.0)
            nc.vector.memset(xpad2[0:K, W - 4:W], 0.0)
            nc.vector.memset(xpad2[K:128, 0:3], 0.0)
            nc.vector.memset(xpad2[K:128, W - 5:W], 0.0)
            nc.sync.dma_start(out=xpad2[0:K, 4:4 + L], in_=xin_d.ap())
            nc.sync.dma_start(out=xpad2[K:128, 3:3 + L], in_=xin_d.ap())

            w1p = singles.tile([128, 5, 2, 128], bf16)
            nc.scalar.dma_start(out=w1p, in_=w1p_d.ap())
            b1sb = singles.tile([128, 2], f32)
            nc.sync.dma_start(out=b1sb, in_=b1t_d.ap())
            # later-needed weights go on the scalar DGE queue so they don't
            # delay xpad2/w1p (conv1's inputs) on the sync queue
            w2blk = singles.tile([48, 2, NO], bf16)
            nc.scalar.dma_start(out=w2blk, in_=w2blk_d.ap())
            wfold = singles.tile([128, 2, G3, NO], bf16)
            nc.scalar.dma_start(out=wfold, in_=wfold_d.ap())
            blk8 = singles.tile([128, 128], bf16)
            nc.gpsimd.dma_start(out=blk8, in_=blk8_d.ap())
            idbf = singles.tile([128, 128], bf16)
            nc.gpsimd.dma_start(out=idbf, in_=iden_d.ap())
            if not _B2_IS_ZERO[0]:
                b2frep = singles.tile([128, NO], f32)
                nc.sync.dma_start(
                    out=b2frep,
                    in_=b2f_d.ap().unsqueeze(0).broadcast_to([128, NO]))
                b2prep = singles.tile([128, NO], f32)
                nc.sync.dma_start(
                    out=b2prep,
                    in_=b2p_d.ap().unsqueeze(0).broadcast_to([128, NO]))
            cst0 = singles.tile([128, 1], f32)
            nc.vector.memset(cst0, 0.0)
            cst1 = singles.tile([128, 1], f32)
            nc.vector.memset(cst1, 1.0)
            cstE = singles.tile([128, 1], f32)
            nc.vector.memset(cstE, EPS)

            # ysq: squashed conv1 output, [ch, l] with 1-col zero pad each side
            ysq = [singles.tile([128, L + 2], bf16, tag=f"ysq{h}",
                                name=f"ysq{h}") for h in range(2)]
            for h in range(2):
                nc.vector.memset(ysq[h][:, 0:1], 0.0)
                nc.vector.memset(ysq[h][:, L + 1:L + 2], 0.0)
            # ytap3[(jj,dh,a), q, l] = ysq_pad[(2q+jj)*8+a, l+dh]
            ytap3 = singles.tile([48, 16, L], bf16)

            def squash_factor(sq_ap, n, key, tagp="sqf", nbufs=4):
                """f = sq/((1+sq)*sqrt(sq+eps)) = sq*exp(-(ln(1+sq)+.5ln(sq+eps)))
                sq_ap: [128, n] f32. Returns f [128, n] f32. ACT+GpSimd only."""
                # l1/e share a ring tag, as do l2/f (l1 dead once t is made,
                # l2 dead once t is made) -- halves the slot count.
                l1 = sqa.tile([128, n], f32, tag=f"{tagp}_a",
                              name=f"l1_{key}", bufs=nbufs)
                nc.scalar.activation(out=l1, in_=sq_ap, func=AF.Ln,
                                     bias=cst1[:, 0:1], scale=1.0)
                l2 = sqa.tile([128, n], f32, tag=f"{tagp}_b",
                              name=f"l2_{key}", bufs=nbufs)
                nc.scalar.activation(out=l2, in_=sq_ap, func=AF.Ln,
                                     bias=cstE[:, 0:1], scale=1.0)
                t = sqa.tile([128, n], f32, tag=f"{tagp}_t",
                             name=f"t_{key}", bufs=nbufs)
                nc.vector.scalar_tensor_tensor(
                    out=t, in0=l2, scalar=0.5, in1=l1, op0=OP.mult, op1=OP.add)
                e = sqa.tile([128, n], f32, tag=f"{tagp}_a",
                             name=f"e_{key}", bufs=nbufs)
                nc.scalar.activation(out=e, in_=t, func=AF.Exp,
                                     bias=cst0[:, 0:1], scale=-1.0)
                f = sqa.tile([128, n], f32, tag=f"{tagp}_b",
                             name=f"f_{key}", bufs=nbufs)
                # GPSIMD cannot read PSUM; conv1's sq lives there -> DVE
                import concourse.bass as _bass
                eng = (nc.vector if sq_ap.space == _bass.MemorySpace.PSUM
                       else nc.gpsimd)
                eng.tensor_tensor(out=f, in0=sq_ap, in1=e, op=OP.mult)
                return f

            # ------------- stage A: conv1 + squash, out ysq [ch, l] -------------
            dmae = [nc.sync]

            ytap_late = []

            def emit_ytap_half(h):
                """DRAM bounce, pipelined per ysq half: write half h to
                ydram, then read back the 6 (jj,dh) row-groups for the 8
                q's of this half. All on the sync HWDGE queue -> FIFO
                ordering makes the reads see the completed writes."""
                nc.sync.dma_start(
                    out=ydram_d.ap()[h * 128:(h + 1) * 128, :], in_=ysq[h])
                ysrc = ydram_d.ap()[h * 128:(h + 1) * 128, :].rearrange(
                    "(q jj a) c -> jj a q c", jj=2, a=8)
                for jj in range(2):
                    for dh in range(G3):
                        r0 = 24 * jj + 8 * dh
                        nc.sync.dma_start(
                            out=ytap3[r0:r0 + 8, 8 * h:8 * h + 8, 0:512],
                            in_=ysrc[jj][:, :, dh:dh + 512])
                        ytap_late.append(
                            (ytap3[r0:r0 + 8, 8 * h:8 * h + 8, 512:L],
                             ysrc[jj][:, :, dh + 512:dh + L]))

            for hf in range(2):
                for lh in range(2):
                    yps = ps.tile([128, 512], f32, tag="ps512", name="yps")
                    for t in range(5):
                        nc.tensor.matmul(
                            yps, lhsT=w1p[:, t, hf, :],
                            rhs=xpad2[:, lh * 512 + 2 * t: lh * 512 + 2 * t + 512],
                            start=(t == 0), stop=(t == 4))
                    ybf = sm.tile([128, 512], bf16, tag="ybf", name="ybf",
                                  bufs=2)
                    nc.scalar.activation(out=ybf, in_=yps, func=AF.Identity,
                                         bias=b1sb[:, hf:hf + 1], scale=1.0)
                    y2 = sm.tile([128, 512], bf16, tag="y2", name="y2",
                                 bufs=1)
                    nc.vector.tensor_mul(y2, ybf, ybf)
                    sqrep = ps.tile([128, 512], f32, tag="ps512", name="sqrep")
                    nc.tensor.matmul(sqrep, lhsT=blk8, rhs=y2,
                                     start=True, stop=True)
                    fA = squash_factor(sqrep, 512, f"A{hf}{lh}",
                                       tagp="c1", nbufs=1)
                    nc.vector.tensor_mul(
                        ysq[hf][:, 1 + lh * 512: 1 + lh * 512 + 512], ybf, fA)
                # ysq half hf complete -> start its ytap3 bounce DMAs now
                emit_ytap_half(hf)
            for out_ap, in_ap in ytap_late:
                nc.sync.dma_start(out=out_ap, in_=in_ap)

            # ------------- state shared across the schedule -------------
            vt = [None] * NSC        # V tiles (vpool ring, 3 live at a time)
            ssb_t = {}               # (sc, j) -> current s in SBUF bf16 [p,a,s]
            f_t = {}                 # (sc, j) -> squash factor [p, s] f32
            et_t = {}                # (sc, j) -> running exp(b) bf16 [p,c,s]
            pps_t = {}               # sc -> pps psum tile
            sps_t = {}               # sc -> last s-op psum tile

            def emit_s0(sc):
                """wfold s0 (uniform-c routing iter 0) -> PSUM -> ssb bf16."""
                s0b = ps_s.tile([128, 2, NO], f32, tag="pss", name=f"s0_{sc}")
                for j in range(2):
                    c = 2 * sc + j
                    first = True
                    for h in range(2):
                        for dh in range(G3):
                            nc.tensor.matmul(
                                s0b[:, j], lhsT=ysq[h][:, c * PL + dh: c * PL + dh + PL],
                                rhs=wfold[:, h, dh, :],
                                start=first, stop=(h == 1 and dh == G3 - 1))
                            first = False
                for j in range(2):
                    ssb = sm.tile([128, ASA, CSA], bf16, tag="ssb",
                                  name=f"ssb0_{sc}_{j}")
                    src = s0b[:, j].rearrange("p (a s) -> p a s", s=CSA)
                    if _B2_IS_ZERO[0]:
                        nc.scalar.activation(
                            out=ssb.rearrange("p a s -> p (a s)"),
                            in_=s0b[:, j], func=AF.Copy, bias=0.0, scale=1.0)
                    else:
                        nc.vector.tensor_tensor(
                            out=ssb, in0=src,
                            in1=b2frep.rearrange("p (a s) -> p a s", s=CSA),
                            op=OP.add)
                    ssb_t[(sc, j)] = ssb

            def emit_squash(sc, j, r):
                """squash pieces for ssb_t[(sc,j)]: sets f_t; r<3: vprev."""
                ssb = ssb_t[(sc, j)]
                # s2 stored [p, s, a] (transposed write on GpSimd) so the
                # a-reduce below reads contiguous runs
                s2 = sqa.tile([128, CSA, ASA], bf16, tag="s2",
                              name=f"s2_{sc}_{j}_{r}", bufs=2)
                nc.gpsimd.tensor_tensor(out=s2.transpose([0, 2, 1]),
                                        in0=ssb, in1=ssb, op=OP.mult)
                sqv = sqa.tile([128, CSA], f32, tag="sqv",
                               name=f"sqv_{sc}_{j}_{r}")
                nc.vector.tensor_reduce(
                    out=sqv, in_=s2, axis=mybir.AxisListType.X, op=OP.add)
                f_t[(sc, j)] = squash_factor(sqv, CSA, f"R{sc}_{j}_{r}")

            def emit_vprev(sc, j, r):
                """vprev = ssb * f (bf16) for the next P-op."""
                # bufs=8: all 8 prologue vprevs stay live until their h1
                vprev = sm.tile([128, ASA, CSA], bf16, tag="vprev",
                                name=f"vprev_{sc}_{j}_{r}", bufs=8)
                nc.vector.tensor_tensor(
                    out=vprev, in0=ssb_t[(sc, j)],
                    in1=f_t[(sc, j)].unsqueeze(1).broadcast_to([128, ASA, CSA]),
                    op=OP.mult)
                return vprev

            vprev_t = {}

            def emit_conv2_one(sc, q, j, split_evac=False):
                """one conv2 matmul + psum evacuation into vt[sc] (ACT;
                alternating ACT/DVE when split_evac, prologue only)."""
                v = vt[sc]
                if True:
                    if True:
                        c = 2 * sc + j
                        vps = ps.tile([128, 512], f32, tag="ps512",
                                      name=f"vps{sc}_{q}_{j}")
                        nc.tensor.matmul(
                            vps, lhsT=ytap3[0:48, q, c * PL:(c + 1) * PL],
                            rhs=w2blk[0:48, :, :], start=True, stop=True)
                        dst = v[:, j, 2 * q:2 * q + 2, :, :].rearrange(
                            "p c a s -> p (c a s)")
                        if _B2_IS_ZERO[0]:
                            if split_evac and (q + j) % 2 == 1:
                                nc.vector.tensor_copy(dst, vps)
                            else:
                                nc.scalar.activation(out=dst, in_=vps,
                                                     func=AF.Copy,
                                                     bias=0.0, scale=1.0)
                        else:
                            nc.vector.scalar_tensor_tensor(
                                out=dst, in0=vps, scalar=1.0,
                                in1=b2prep.unsqueeze(1).broadcast_to(
                                    [128, 2, NO]).rearrange("p c s -> p (c s)"),
                                op0=OP.mult, op1=OP.add)

            conv2_q = []   # pending (sc, q, j) conv2 work, dripped into
                           # PE gaps so evacuation spreads across the round

            def queue_conv2(sc, q0, q1):
                for q in range(q0, q1):
                    for j in range(2):
                        conv2_q.append((sc, q, j))

            def drip(n):
                for _ in range(min(n, len(conv2_q))):
                    sc, q, j = conv2_q.pop(0)
                    emit_conv2_one(sc, q, j)

            def emit_conv2(sc, q0, q1, split_evac=False):
                for j in range(2):
                    for q in range(q0, q1):
                        emit_conv2_one(sc, q, j, split_evac)

            def alloc_vt(sc):
                vt[sc] = vpool.tile([128, 2, CP, ASA, CSA], bf16,
                                    tag="vt", name=f"vt{sc}")

            def emit_h1(sc, r):
                """mt build + P-op + exp + zt per j. vprev must be ready."""
                v = vt[sc]
                pps = ps_p.tile([128, 2, CP, CSA], f32, tag="pps",
                                name=f"pps_{sc}_{r}")
                pps_t[sc] = pps
                for j in range(2):
                    mt = mtx.tile([128, ASA, CP, CSA], bf16, tag="mtx",
                                  name=f"mt_{sc}_{r}_{j}")
                    nc.vector.tensor_tensor(
                        out=mt, in0=v[:, j].transpose([0, 2, 1, 3]),
                        in1=vprev_t[(sc, j)].unsqueeze(2).broadcast_to(
                            [128, ASA, CP, CSA]),
                        op=OP.mult)
                    drip(2)
                    for ai in range(ASA):
                        nc.tensor.matmul(
                            pps[:, j], lhsT=idbf,
                            rhs=mt[:, ai].rearrange("p c s -> p (c s)"),
                            start=(ai == 0), stop=(ai == ASA - 1))
                    # bufs=6: r1's ep (= et) is read again by r2's et2 multiply
                    ep = sm.tile([128, CP, CSA], bf16, tag="ep",
                                 name=f"ep_{sc}_{r}_{j}", bufs=6)
                    nc.scalar.activation(
                        out=ep.rearrange("p c s -> p (c s)"),
                        in_=pps[:, j].rearrange("p c s -> p (c s)"),
                        func=AF.Exp, bias=cst0[:, 0:1], scale=1.0)
                    if r == 1:
                        et = ep
                    else:
                        et = sm.tile([128, CP, CSA], bf16, tag="et2",
                                     name=f"et2_{sc}_{r}_{j}")
                        nc.gpsimd.tensor_tensor(
                            out=et, in0=et_t[(sc, j)], in1=ep, op=OP.mult)
                    et_t[(sc, j)] = et
                    zt = sqa.tile([128, CP], f32, tag="zt",
                                  name=f"zt_{sc}_{r}_{j}")
                    nc.vector.tensor_reduce(
                        out=zt, in_=et, axis=mybir.AxisListType.X, op=OP.add)
                    et_t[("z", sc, j)] = zt

            def emit_h2(sc, r):
                """rz + ct + mt2 + s-op + ssb evac + squash (+final v2/DMA)."""
                v = vt[sc]
                final = (r == 2)
                rz_j, ct_j = {}, {}
                for j in range(2):
                    rz = sqa.tile([128, CP], f32, tag="rz",
                                  name=f"rz_{sc}_{r}_{j}")
                    nc.vector.reciprocal(rz, et_t[("z", sc, j)])
                    rz_j[j] = rz
                for j in range(2):
                    ct = sm.tile([128, CP, CSA], bf16, tag="ct",
                                 name=f"ct_{sc}_{r}_{j}", bufs=2)
                    nc.vector.tensor_tensor(
                        out=ct, in0=et_t[(sc, j)],
                        in1=rz_j[j].unsqueeze(2).broadcast_to([128, CP, CSA]),
                        op=OP.mult)
                    ct_j[j] = ct
                sps = ps_s.tile([128, 2, NO], f32, tag="pss",
                              name=f"sps_{sc}_{r}")
                sps_t[sc] = sps
                for j in range(2):
                    mt2 = mtx.tile([128, CP, ASA, CSA], bf16, tag="mtx",
                                   name=f"mt2_{sc}_{r}_{j}")
                    nc.vector.tensor_tensor(
                        out=mt2, in0=v[:, j],
                        in1=ct_j[j].unsqueeze(2).broadcast_to(
                            [128, CP, ASA, CSA]),
                        op=OP.mult)
                    drip(2)
                    for cpi in range(CP):
                        nc.tensor.matmul(
                            sps[:, j], lhsT=idbf,
                            rhs=mt2[:, cpi].rearrange("p a s -> p (a s)"),
                            start=(cpi == 0), stop=(cpi == CP - 1))
                    ssb = sm.tile([128, ASA, CSA], bf16, tag="ssb",
                                  name=f"ssb_{sc}_{r}_{j}")
                    nc.scalar.activation(
                        out=ssb.rearrange("p a s -> p (a s)"),
                        in_=sps[:, j], func=AF.Copy, bias=0.0, scale=1.0)
                    ssb_t[(sc, j)] = ssb
                    emit_squash(sc, j, r + 1)
                    if not final:
                        vprev_t[(sc, j)] = emit_vprev(sc, j, r + 1)
                    else:
                        c = 2 * sc + j
                        v2 = vout.tile([128, CSA, ASA], f32, tag="v2",
                                       name=f"v2_{sc}_{j}")
                        nc.gpsimd.tensor_tensor(
                            out=v2.transpose([0, 2, 1]), in0=ssb_t[(sc, j)],
                            in1=f_t[(sc, j)].unsqueeze(1).broadcast_to(
                                [128, ASA, CSA]),
                            op=OP.mult)
                        (nc.scalar if sc % 2 else nc.gpsimd).dma_start(
                            out=out_v[c * PL:(c + 1) * PL], in_=v2)

            # ------------- prologue: s0 prefill + first conv2s -------------
            def s0_and_squash(sc):
                emit_s0(sc)
                for j in range(2):
                    emit_squash(sc, j, 1)
                    vprev_t[(sc, j)] = emit_vprev(sc, j, 1)

            s0_and_squash(0)
            s0_and_squash(1)
            for sc in range(3):
                alloc_vt(sc)
            # conv2(0) evacs on ACT only (keeps DVE free for mt(0));
            # h1(0,1) is emitted right after -- its mt(0,j0) only waits on
            # the j0-half evacs via subtile deps.
            emit_conv2(0, 0, 16)

            # ------------- routing: flat 2-deep rolling pipeline -------------
            emit_h1(0, 1)
            emit_conv2(1, 0, 16, split_evac=True)
            emit_h2(0, 1)
            s0_and_squash(2)
            queue_conv2(2, 0, 16)
            emit_h1(1, 1)
            drip(4)
            emit_h2(1, 1)
            drip(4)
            emit_h1(0, 2)
            drip(4)
            emit_h1(1, 2)
            drip(len(conv2_q))
            emit_h2(0, 2)
            # v(0)'s last read (mt2 of 0,r2) is emitted; sc3 slot reuse safe
            alloc_vt(3)
            queue_conv2(3, 0, 16)
            s0_and_squash(3)
            emit_h1(2, 1)
            emit_h2(1, 2)
            # all conv2(3) must be emitted before h1(3,1): its P-op would
            # otherwise head-of-line block the PE FIFO ahead of the conv2
            # matmuls whose evacs mt(3) waits on (deadlock).
            drip(len(conv2_q))
            emit_h1(3, 1)
            emit_h2(2, 1)
            emit_h1(2, 2)
            emit_h2(3, 1)
            emit_h1(3, 2)
            emit_h2(2, 2)
            emit_h2(3, 2)
    nc.compile()
    return nc


def _prep_weights(w1, b1, w2, b2):
    import ml_dtypes
    w1 = np.asarray(w1, np.float32)
    w2 = np.asarray(w2, np.float32)
    b1 = np.asarray(b1, np.float32)
    b2 = np.asarray(b2, np.float32)
    # o-permutation: column order (a, s): col a*16+s <- orig o = s*16+a
    a_i, s_i = np.meshgrid(np.arange(ASA), np.arange(CSA), indexing="ij")
    perm = (s_i * ASA + a_i).reshape(-1)
    w2m = w2[:, 0, :, :]                             # [o, dh, ap]
    w2p = w2m[perm]                                  # [o'=(a,s), dh, ap]

    # conv1 tap pairs: rows 0:64 = tap 2p, rows 64:128 = tap 2p+1 (p=4: zero)
    w1p = np.zeros((128, 5, 2, 128), np.float32)
    for hf in range(2):
        wt = w1[hf * 128:(hf + 1) * 128]             # [m, k, t]
        for p in range(4):
            w1p[0:K, p, hf, :] = wt[:, :, 2 * p].T
            w1p[K:128, p, hf, :] = wt[:, :, 2 * p + 1].T
        w1p[0:K, 4, hf, :] = wt[:, :, 8].T
    b1t = np.zeros((128, 2), np.float32)
    for hf in range(2):
        b1t[:, hf] = b1[hf * 128:(hf + 1) * 128]

    w2blk = np.zeros((48, 2, NO), np.float32)
    for jj in range(2):
        for dh in range(G3):
            for ap in range(APd):
                w2blk[24 * jj + dh * 8 + ap, jj, :] = w2p[:, dh, ap]

    wfold = np.zeros((128, 2, G3, NO), np.float32)
    co = np.arange(256)
    for dh in range(G3):
        wf = (w2p[:, dh, :].T[co % 8, :] / float(CSA))
        wfold[:, 0, dh, :] = wf[0:128]
        wfold[:, 1, dh, :] = wf[128:256]

    blk8 = np.kron(np.eye(16, dtype=np.float32), np.ones((8, 8), np.float32))

    return {
        "w1p": w1p.astype(ml_dtypes.bfloat16),
        "b1t": b1t,
        "w2blk": w2blk.astype(ml_dtypes.bfloat16),
        "wfold": wfold.astype(ml_dtypes.bfloat16),
        "blk8": blk8.astype(ml_dtypes.bfloat16),
        "iden": np.eye(128, dtype=np.float32).astype(ml_dtypes.bfloat16),
        "b2p": np.ascontiguousarray(b2[perm]),
        "b2f": np.ascontiguousarray(2.0 * b2[perm]).astype(np.float32),
    }


def kernel(x, w1, b1, w2, b2):
    import ml_dtypes
    from concourse.bass_utils import run_bass_kernel_spmd

    x = np.asarray(x, np.float32)
    _B2_IS_ZERO[0] = bool(np.all(np.asarray(b2) == 0.0))
    if "nc" not in _CACHE:
        _CACHE["nc"] = _build_nc()
    nc = _CACHE["nc"]
    wmaps = _prep_weights(w1, b1, w2, b2)
    xbf = x.astype(ml_dtypes.bfloat16)
    in_maps = []
    for b in range(NCORES):
        m = {"xin": np.ascontiguousarray(xbf[b])}
        m.update(wmaps)
        in_maps.append(m)
    trace = bool(int(os.environ.get("KERNEL_TRACE", "0")))
    res = run_bass_kernel_spmd(
        nc, in_maps, core_ids=list(range(NCORES)), trace=trace)
    if trace:
        _CACHE["last_exec_time_ns"] = res.exec_time_ns
        _CACHE["last_trace"] = res.instructions_and_trace
    out = np.stack([r["out"] for r in res.results])   # [8, 16384, 16]
    return out.astype(np.float32)
